# revision 21
# baseline (speedup 1.0000x reference)
"""Trainium2 kernel for nn_CustomizedMoGPositionwiseFF (moe_routing).

Strategy (expert-parallel, per the sharding hint):
  - 32 (group, expert) FFN pairs are sharded across 8 NeuronCores (4 each).
  - Routing (group top-2 gate + per-group inner top-2 gate) is computed on
    host at call time; tokens are dispatched (gathered) per expert into the
    per-core shards -- data-dependent sharding, compiled into the NEFF.
  - Each core runs both FFN matmuls + relu for its 4 experts over the tokens
    routed to them, reading each expert weight exactly once (memory regime).
  - Weights/activations ship as fp8e4 (TRN E4M3, max 240) with power-of-two
    scales; matmuls run in DoubleRow perf mode (2 fp8 k-tiles per pass, 2x
    PE throughput, f32 PSUM accumulation):
        zt = 4*z,  w1q = 8*W1   -> psum1 = 32*(z@W1)
        h  = relu(psum1) fp8    (= 32*relu(z@W1), max ~80 < 240)
        w2q = 16*W2             -> psum2 = 512*(h@W2)
        u  = psum2/16 fp8       (= 32*u, max ~50 < 240); host divides by 32.
  - Host applies the cheap O(N*D) combine: iw/b2 scaling, scatter-add of the
    two expert contributions per (token, group), per-group post-layernorm,
    group top-2 mixture, and the outer residual.

Device layouts are partition-major; the DoubleRow pair dim is always the
middle AP dim ([128, 2, cols]): zt/w1 are dt-paired, h/w2 are ht-paired
(w2 stored dt-major so each output d-tile reads a contiguous slab).
"""

import os
import numpy as np

# Model dims (hardcoded per the contract; match the reference problem)
B, T, D, H = 2, 1024, 512, 2048
G, E, GK, EK = 4, 8, 2, 2
EPS = 1e-5
N = B * T
P = 128
DT = D // P    # 4 d-tiles
HT = H // P    # 16 h-tiles
NCORES = 8
SLOTS = (G * E) // NCORES  # 4 experts per core
CAP_GRAN = 4               # capacity granularity (tokens)
L1_CHUNK = 512             # moving-dim chunk for both layers

# fp8 scales (powers of two; see module docstring)
SZ = 4.0    # z
S1 = 8.0    # W1
S2 = 16.0   # W2
SU = 16.0   # psum2 -> u divisor on device
UDQ = (SZ * S1 * S2) / SU  # device u = UDQ * true u; host divides (=32)
F8MAX = 240.0              # TRN E4M3 max normal (inf at 256)

# 1 = split relu between scalar(ACT) and vector(DVE); 0 = all DVE
RELU_SPLIT = int(os.environ.get("KERNEL_RELU_SPLIT", "1"))

_nc_cache = {}
LAST_RESULTS = None       # test harness can inspect (BassKernelResults)


def _ensure_ntff_hook():
    """Register antenv.axon_hooks with the ctypes NTFF profile hook if the
    container's antenv package lacks it (mirrors trn_agent_boot.trn_boot).
    Makes trace=True work; degrades to hook=None when the .so is absent."""
    try:
        from antenv.axon_hooks import get_axon_ntff_profile_hook  # noqa: F401
        return
    except ImportError:
        pass
    import sys
    import types
    import contextlib
    import ctypes

    mod = types.ModuleType("antenv.axon_hooks")
    _state = {"hook": None}

    def set_axon_ntff_profile_hook(h):
        _state["hook"] = h

    def get_axon_ntff_profile_hook():
        return _state["hook"]

    mod.set_axon_ntff_profile_hook = set_axon_ntff_profile_hook
    mod.get_axon_ntff_profile_hook = get_axon_ntff_profile_hook

    so_path = "/opt/axon/libaxon_pjrt.so"
    hook = None
    if os.path.exists(so_path):
        try:
            lib = ctypes.CDLL(so_path)
            if hasattr(lib, "axon_start_nrt_profile"):
                lib.axon_start_nrt_profile.argtypes = [
                    ctypes.POINTER(ctypes.c_int64), ctypes.c_size_t]
                lib.axon_start_nrt_profile.restype = ctypes.c_int64
                lib.axon_stop_nrt_profile.argtypes = [ctypes.c_char_p]
                lib.axon_stop_nrt_profile.restype = ctypes.c_int64

                @contextlib.contextmanager
                def _hook(output_dir, device_ids):
                    import jax
                    jax.devices()
                    if device_ids:
                        ids = (ctypes.c_int64 * len(device_ids))(*device_ids)
                        rc = lib.axon_start_nrt_profile(ids, len(device_ids))
                    else:
                        rc = lib.axon_start_nrt_profile(None, 0)
                    if rc != 0:
                        raise RuntimeError(f"axon_start_nrt_profile rc={rc}")
                    try:
                        yield
                    finally:
                        n = lib.axon_stop_nrt_profile(str(output_dir).encode())
                        print(f"ntff profile: {n} file(s) -> {output_dir}")

                hook = _hook
        except Exception:
            hook = None
    _state["hook"] = hook
    import antenv
    sys.modules["antenv.axon_hooks"] = mod
    antenv.axon_hooks = mod


def _round_up(x, m):
    return ((x + m - 1) // m) * m


def _routing(inp, ln_g, ln_b, wg_group, wg_inner):
    """Replicate the reference gating bit-for-bit on jax-cpu.

    Returns gi [N,GK] group ids, gsc [N,GK] group softmax, z [N,D] f32,
    eis/escs: per-group inner top-k ids/softmax ([N,EK] each).
    """
    import jax
    import jax.numpy as jnp

    cpu = jax.devices("cpu")[0]
    with jax.default_device(cpu):
        x = jnp.asarray(np.asarray(inp, np.float32)).reshape(-1, D)
        gl = x @ jnp.asarray(np.asarray(wg_group, np.float32))
        gv, gi = jax.lax.top_k(gl, GK)
        gsc = jax.nn.softmax(gv, axis=-1)
        m = jnp.mean(x, axis=-1, keepdims=True)
        xc = x - m
        v = jnp.mean(xc * xc, axis=-1, keepdims=True)
        z = xc * jax.lax.rsqrt(v + EPS) * jnp.asarray(np.asarray(ln_g, np.float32)) \
            + jnp.asarray(np.asarray(ln_b, np.float32))
        wgi = jnp.asarray(np.asarray(wg_inner, np.float32))
        eis, escs = [], []
        for g in range(G):
            l = z @ wgi[g]
            ev, ei = jax.lax.top_k(l, EK)
            esc = jax.nn.softmax(ev, axis=-1)
            eis.append(np.asarray(ei))
            escs.append(np.asarray(esc))
    return np.asarray(gi), np.asarray(gsc), np.asarray(z), eis, escs


def _build_nc(Cs, has_b1=False):
    """Build the SPMD Bass program for per-slot capacities Cs (uniform across cores)."""
    import concourse.bass as bass
    import concourse.bacc as bacc
    import concourse.tile as tile
    from concourse import mybir

    f32 = mybir.dt.float32
    fp8 = mybir.dt.float8e4
    Relu = mybir.ActivationFunctionType.Relu
    DR = mybir.MatmulPerfMode.DoubleRow
    max_op = mybir.AluOpType.max
    mult_op = mybir.AluOpType.mult

    CT = int(sum(Cs))
    MAXW = int(max(min(int(c), L1_CHUNK) for c in Cs))
    offs = np.concatenate([[0], np.cumsum(Cs)]).astype(int)

    nc = bacc.Bacc("TRN2", target_bir_lowering=False)
    # all DRAM layouts are partition-major so every DMA is 128 dense lines.
    # w1 is h-half-major ([P, half, dt, H/2]) so slot0's first half (all dt,
    # ht 0..7) is one contiguous-line DMA and layer 1 can start on it.
    zt_d = nc.declare_dram_parameter("zt", [P, DT, CT], fp8, isOutput=False)
    w1_d = nc.declare_dram_parameter("w1", [SLOTS, P, 2 * DT, H // 2], fp8, isOutput=False)
    w2_d = nc.declare_dram_parameter("w2", [SLOTS, P, DT * HT, P], fp8, isOutput=False)
    if has_b1:
        b1_d = nc.declare_dram_parameter("b1", [P, SLOTS * HT], f32, isOutput=False)
    u_d = nc.declare_dram_parameter("u", [P, DT, CT], fp8, isOutput=True)

    with tile.TileContext(nc) as tc:
        with tc.tile_pool(name="consts", bufs=1) as consts, \
             tc.tile_pool(name="hpool", bufs=2) as hpool, \
             tc.tile_pool(name="hpsum", bufs=4, space="PSUM") as hpsum, \
             tc.tile_pool(name="upsum", bufs=3, space="PSUM") as upsum, \
             tc.tile_pool(name="usb", bufs=3) as usb:

            zt_sb = consts.tile([P, DT, CT], fp8, tag="zt")
            zero_sb = consts.tile([P, MAXW], f32, tag="zero")
            sixt_sb = consts.tile([P, MAXW], f32, tag="sixt")
            nc.vector.memset(zero_sb[:, :], 0.0)
            nc.vector.memset(sixt_sb[:, :], 1.0 / SU)
            if has_b1:
                b1_sb = consts.tile([P, SLOTS * HT], f32, tag="b1")
            w1_sb, w2_sb = [], []
            for s in range(SLOTS):
                w1_sb.append(consts.tile([P, 2 * DT, H // 2], fp8, tag=f"w1_{s}", name=f"w1s_{s}"))
                w2_sb.append(consts.tile([P, DT * HT, P], fp8, tag=f"w2_{s}", name=f"w2s_{s}"))

            # ---- resident loads. Measured (v2/v5/v7 traces): data only
            # starts flowing ~9us in (instruction fetch + trigger latency);
            # each HWDGE ring then streams 160-190 GB/s serially in queue
            # order; the SWDGE ring starts later and runs ~3x slower. So:
            # weights + zt go ONLY on the two HWDGE rings, every piece a
            # ~4KB-line half so both rings advance in lock-step with
            # first-use order (h-half of w1 feeds the software-pipelined
            # half-L1 emits below); SWDGE carries only the u stores. A late
            # piece stalls the PE, which also re-throttles the HAM p-state.
            # (No PE warm-up matmuls: measured throttle_active grows with
            # total PE activity -- an activity/power cap -- so dummy matmuls
            # would spend duty budget on zeros.)
            nc.sync.dma_start(zt_sb[:, :, :], zt_d[:, :, :])
            nc.scalar.dma_start(w1_sb[0][:, 0:DT, :], w1_d[0][:, 0:DT, :])
            nc.sync.dma_start(w1_sb[0][:, DT:2 * DT, :], w1_d[0][:, DT:2 * DT, :])
            for s in range(SLOTS):
                if s + 1 < SLOTS:
                    nc.scalar.dma_start(
                        w1_sb[s + 1][:, 0:DT, :], w1_d[s + 1][:, 0:DT, :])
                nc.scalar.dma_start(
                    w2_sb[s][:, 2 * HT:, :], w2_d[s][:, 2 * HT:, :])
                nc.sync.dma_start(
                    w2_sb[s][:, 0:2 * HT, :], w2_d[s][:, 0:2 * HT, :])
                if s + 1 < SLOTS:
                    nc.sync.dma_start(
                        w1_sb[s + 1][:, DT:2 * DT, :], w1_d[s + 1][:, DT:2 * DT, :])
            if has_b1:
                nc.scalar.dma_start(b1_sb[:, :], b1_d[:, :])

            # ---- compute, software-pipelined: emit the NEXT chunk's first
            # half of layer 1 before this chunk's layer 2, so the relu
            # engines (DVE/ACT) get ~2us of PE runway and layer-2 matmuls
            # never wait on h (the v7 trace showed 32us of PE sem-waits on
            # the relu producer sems).
            chunks = []
            for s in range(SLOTS):
                C = int(Cs[s])
                off = int(offs[s])
                for c0 in range(0, C, L1_CHUNK):
                    chunks.append((s, off + c0, min(L1_CHUNK, C - c0)))
            h_tiles = {}

            def emit_l1(ci, hts):
                s, base, W = chunks[ci]
                if ci not in h_tiles:
                    h_tiles[ci] = hpool.tile(
                        [P, HT, W], fp8, tag="h", name=f"h_{ci}")
                h_sb = h_tiles[ci]
                for ht in hts:
                    hf, h8 = ht // 8, ht % 8
                    ph = hpsum.tile([P, W], f32, tag="ph", name=f"ph_{ci}_{ht}")
                    for j, dt in enumerate((0, 2)):
                        nc.tensor.matmul(
                            ph[:, :],
                            w1_sb[s][:, hf * DT + dt: hf * DT + dt + 2,
                                     h8 * P:(h8 + 1) * P],
                            zt_sb[:, dt:dt + 2, base:base + W],
                            start=(j == 0),
                            stop=(j == 1),
                            perf_mode=DR,
                        )
                    # relu split: 9 on ACT (even ht + ht15), 7 on DVE
                    if has_b1:
                        nc.scalar.activation(
                            h_sb[:, ht, :], ph[:, :], Relu,
                            bias=b1_sb[:, s * HT + ht: s * HT + ht + 1],
                        )
                    elif RELU_SPLIT and (ht % 2 == 0 or ht == HT - 1):
                        nc.scalar.activation(h_sb[:, ht, :], ph[:, :], Relu)
                    else:
                        nc.vector.tensor_tensor(
                            h_sb[:, ht, :], ph[:, :], zero_sb[:, :W], max_op)

            def emit_l2(ci):
                s, base, W = chunks[ci]
                h_sb = h_tiles[ci]
                u_sb = usb.tile([P, DT, W], fp8, tag="u", name=f"u_{ci}")
                for dt in range(DT):
                    pu = upsum.tile([P, W], f32, tag="pu", name=f"pu_{ci}_{dt}")
                    for j, ht in enumerate(range(0, HT, 2)):
                        nc.tensor.matmul(
                            pu[:, :],
                            w2_sb[s][:, dt * HT + ht: dt * HT + ht + 2, :],
                            h_sb[:, ht:ht + 2, :],
                            start=(j == 0),
                            stop=(j == HT // 2 - 1),
                            perf_mode=DR,
                        )
                    nc.vector.tensor_tensor(
                        u_sb[:, dt, :], pu[:, :], sixt_sb[:, :W], mult_op)
                # one batched output DMA per chunk on the SWDGE path
                nc.gpsimd.dma_start(u_d[:, :, base:base + W], u_sb[:, :, :])

            emit_l1(0, range(HT))
            for ci in range(len(chunks)):
                if ci + 1 < len(chunks):
                    emit_l1(ci + 1, range(0, HT // 2))
                emit_l2(ci)
                if ci + 1 < len(chunks):
                    emit_l1(ci + 1, range(HT // 2, HT))
    nc.compile()
    return nc


def _get_nc(Cs, has_b1):
    key = (tuple(int(c) for c in Cs), bool(has_b1))
    if key not in _nc_cache:
        _nc_cache[key] = _build_nc(key[0], key[1])
    return _nc_cache[key]


def _q8(x, scale, f8):
    """Host-side quantize x*scale to TRN e4m3 (clip to +-240)."""
    return np.clip(np.asarray(x, np.float32) * scale, -F8MAX, F8MAX).astype(f8)


def kernel(inp, ln_g, ln_b, wg_group, wg_inner, W1, b1, W2, b2, gln_g, gln_b):
    global LAST_RESULTS
    import jax
    import jax.numpy as jnp
    import ml_dtypes

    inp = np.asarray(inp)
    in_dtype = inp.dtype
    f8 = ml_dtypes.float8_e4m3

    # ---- 1. routing on host (bit-exact replica of the reference gates)
    gi, gsc, z, eis, escs = _routing(inp, ln_g, ln_b, wg_group, wg_inner)

    # token lists per (g, e)
    tok_lists, scale_lists = {}, {}
    for g in range(G):
        in_g = (gi == g).any(axis=1)
        S_g = np.nonzero(in_g)[0]
        ei, esc = eis[g], escs[g]
        for e in range(E):
            sel = ei[S_g] == e           # [|S_g|, EK]
            has = sel.any(axis=1)
            toks = S_g[has]
            w = (esc[S_g] * sel).sum(axis=1)[has]
            tok_lists[(g, e)] = toks
            scale_lists[(g, e)] = w.astype(np.float32)

    # ---- 2. balanced assignment of the 32 pairs to (core, slot)
    pairs = [(g, e) for g in range(G) for e in range(E)]
    pairs.sort(key=lambda p: -len(tok_lists[p]))
    assign = {}           # (core, slot) -> (g, e)
    Cs = []
    for s in range(SLOTS):
        rank = pairs[s * NCORES:(s + 1) * NCORES]
        Cs.append(max(CAP_GRAN, _round_up(max(len(tok_lists[p]) for p in rank), CAP_GRAN)))
        for c, p in enumerate(rank):
            assign[(c, s)] = p
    CT = int(sum(Cs))
    offs = np.concatenate([[0], np.cumsum(Cs)]).astype(int)

    # ---- 3. build per-core input maps (fp8 with power-of-two scales)
    W1n = np.asarray(W1, np.float32)
    W2n = np.asarray(W2, np.float32)
    b1n = np.asarray(b1, np.float32)
    b2n = np.asarray(b2, np.float32)
    has_b1 = bool(np.any(b1n))
    z_q = _q8(z, SZ, f8)

    in_maps = []
    for c in range(NCORES):
        zt_np = np.zeros((P, DT, CT), f8)
        w1_np = np.empty((SLOTS, P, 2 * DT, H // 2), f8)
        w2_np = np.empty((SLOTS, P, DT * HT, P), f8)
        if has_b1:
            b1_np = np.empty((P, SLOTS * HT), np.float32)
            b1_v = b1_np.reshape(P, SLOTS, HT)
        for s in range(SLOTS):
            g, e = assign[(c, s)]
            toks = tok_lists[(g, e)]
            n = len(toks)
            off = offs[s]
            # z^T tile: [tok, d] -> [p, dt, c]
            zt_np[:, :, off:off + n] = z_q[toks].T.reshape(DT, P, n).transpose(1, 0, 2)
            # W1 [d, h] -> [p, half*DT+dt, h%1024] (h-half-major)
            w1_np[s] = (
                _q8(W1n[g, e], S1, f8)
                .reshape(DT, P, 2, H // 2).transpose(1, 2, 0, 3)
                .reshape(P, 2 * DT, H // 2)
            )
            # W2 [h, d] -> [p, dt, ht, dc] (dt-major) -> [p, dt*ht, dc]
            w2_np[s] = (
                _q8(W2n[g, e], S2, f8)
                .reshape(HT, P, DT, P).transpose(1, 2, 0, 3).reshape(P, DT * HT, P)
            )
            if has_b1:
                b1_v[:, s, :] = (SZ * S1) * b1n[g, e].reshape(HT, P).T
        m = {"zt": zt_np, "w1": w1_np, "w2": w2_np}
        if has_b1:
            m["b1"] = b1_np
        in_maps.append(m)

    # ---- 4. compile + run on the 8 NeuronCores
    _ensure_ntff_hook()
    from concourse.bass_utils import run_bass_kernel_spmd

    nc = _get_nc(Cs, has_b1=has_b1)
    res = run_bass_kernel_spmd(
        nc, in_maps, core_ids=list(range(NCORES)),
        trace=bool(int(os.environ.get("KERNEL_TRACE", "0"))),
    )
    LAST_RESULTS = res

    # ---- 5. host combine
    moe = np.zeros((G, N, D), np.float32)
    for c in range(NCORES):
        # u layout [p, dt, c] -> u^T[d, c] -> [CT, D]; device u = UDQ * u
        u = (
            np.asarray(res.results[c]["u"], np.float32)
            .transpose(1, 0, 2).reshape(D, CT).T
        )
        for s in range(SLOTS):
            g, e = assign[(c, s)]
            toks = tok_lists[(g, e)]
            n = len(toks)
            w = scale_lists[(g, e)]
            contrib = u[offs[s]:offs[s] + n] * (w / UDQ)[:, None] \
                + w[:, None] * b2n[g, e][None, :]
            np.add.at(moe[g], toks, contrib)

    cpu = jax.devices("cpu")[0]
    with jax.default_device(cpu):
        zj = jnp.asarray(z)
        gi_j = jnp.asarray(gi)
        gsc_j = jnp.asarray(gsc)
        gw_dense = jnp.sum(
            jax.nn.one_hot(gi_j, G, dtype=jnp.float32) * gsc_j[..., None], axis=-2
        )  # [N, G]
        out = jnp.zeros((N, D), jnp.float32)
        gg = jnp.asarray(np.asarray(gln_g, np.float32))
        gb = jnp.asarray(np.asarray(gln_b, np.float32))
        for g in range(G):
            t = zj + jnp.asarray(moe[g])
            m = jnp.mean(t, axis=-1, keepdims=True)
            tc_ = t - m
            v = jnp.mean(tc_ * tc_, axis=-1, keepdims=True)
            y = tc_ * jax.lax.rsqrt(v + EPS) * gg[g] + gb[g]
            out = out + gw_dense[:, g:g + 1] * y
        result = np.asarray(out).reshape(B, T, D) + np.asarray(inp, np.float32)

    return result.astype(in_dtype)


# revision 23
# speedup vs baseline: 1.1757x; 1.1757x over previous
"""Trainium2 kernel for nn_CustomizedMoGPositionwiseFF (moe_routing).

Strategy (expert-parallel, per the sharding hint):
  - 32 (group, expert) FFN pairs are sharded across 8 NeuronCores (4 each).
  - Routing (group top-2 gate + per-group inner top-2 gate) is computed on
    host at call time; tokens are dispatched (gathered) per expert into the
    per-core shards -- data-dependent sharding, compiled into the NEFF.
  - Each core runs both FFN matmuls + relu for its 4 experts over the tokens
    routed to them, reading each expert weight exactly once (memory regime).
  - Weights/activations ship as fp8e4 (TRN E4M3, max 240) with power-of-two
    scales; matmuls run in DoubleRow perf mode (2 fp8 k-tiles per pass, 2x
    PE throughput, f32 PSUM accumulation):
        zt = 4*z,  w1q = 8*W1   -> psum1 = 32*(z@W1)
        h  = relu(psum1) fp8    (= 32*relu(z@W1), max ~80 < 240)
        w2q = 16*W2             -> psum2 = 512*(h@W2)
        u  = psum2/16 fp8       (= 32*u, max ~50 < 240); host divides by 32.
  - Host applies the cheap O(N*D) combine: iw/b2 scaling, scatter-add of the
    two expert contributions per (token, group), per-group post-layernorm,
    group top-2 mixture, and the outer residual.

Device layouts are partition-major; the DoubleRow pair dim is always the
middle AP dim ([128, 2, cols]): zt/w1 are dt-paired, h/w2 are ht-paired
(w2 stored dt-major so each output d-tile reads a contiguous slab).
"""

import os
import numpy as np

# Model dims (hardcoded per the contract; match the reference problem)
B, T, D, H = 2, 1024, 512, 2048
G, E, GK, EK = 4, 8, 2, 2
EPS = 1e-5
N = B * T
P = 128
DT = D // P    # 4 d-tiles
HT = H // P    # 16 h-tiles
NCORES = 8
SLOTS = (G * E) // NCORES  # 4 experts per core
CAP_GRAN = 4               # capacity granularity (tokens)
L1_CHUNK = 512             # moving-dim chunk for both layers

# fp8 scales (powers of two; see module docstring)
SZ = 4.0    # z
S1 = 8.0    # W1
S2 = 16.0   # W2
SU = 16.0   # psum2 -> u divisor on device
UDQ = (SZ * S1 * S2) / SU  # device u = UDQ * true u; host divides (=32)
F8MAX = 240.0              # TRN E4M3 max normal (inf at 256)

# 1 = split relu between scalar(ACT) and vector(DVE); 0 = all DVE
RELU_SPLIT = int(os.environ.get("KERNEL_RELU_SPLIT", "1"))

_nc_cache = {}
LAST_RESULTS = None       # test harness can inspect (BassKernelResults)


def _ensure_ntff_hook():
    """Register antenv.axon_hooks with the ctypes NTFF profile hook if the
    container's antenv package lacks it (mirrors trn_agent_boot.trn_boot).
    Makes trace=True work; degrades to hook=None when the .so is absent."""
    try:
        from antenv.axon_hooks import get_axon_ntff_profile_hook  # noqa: F401
        return
    except ImportError:
        pass
    import sys
    import types
    import contextlib
    import ctypes

    mod = types.ModuleType("antenv.axon_hooks")
    _state = {"hook": None}

    def set_axon_ntff_profile_hook(h):
        _state["hook"] = h

    def get_axon_ntff_profile_hook():
        return _state["hook"]

    mod.set_axon_ntff_profile_hook = set_axon_ntff_profile_hook
    mod.get_axon_ntff_profile_hook = get_axon_ntff_profile_hook

    so_path = "/opt/axon/libaxon_pjrt.so"
    hook = None
    if os.path.exists(so_path):
        try:
            lib = ctypes.CDLL(so_path)
            if hasattr(lib, "axon_start_nrt_profile"):
                lib.axon_start_nrt_profile.argtypes = [
                    ctypes.POINTER(ctypes.c_int64), ctypes.c_size_t]
                lib.axon_start_nrt_profile.restype = ctypes.c_int64
                lib.axon_stop_nrt_profile.argtypes = [ctypes.c_char_p]
                lib.axon_stop_nrt_profile.restype = ctypes.c_int64

                @contextlib.contextmanager
                def _hook(output_dir, device_ids):
                    import jax
                    jax.devices()
                    if device_ids:
                        ids = (ctypes.c_int64 * len(device_ids))(*device_ids)
                        rc = lib.axon_start_nrt_profile(ids, len(device_ids))
                    else:
                        rc = lib.axon_start_nrt_profile(None, 0)
                    if rc != 0:
                        raise RuntimeError(f"axon_start_nrt_profile rc={rc}")
                    try:
                        yield
                    finally:
                        n = lib.axon_stop_nrt_profile(str(output_dir).encode())
                        print(f"ntff profile: {n} file(s) -> {output_dir}")

                hook = _hook
        except Exception:
            hook = None
    _state["hook"] = hook
    import antenv
    sys.modules["antenv.axon_hooks"] = mod
    antenv.axon_hooks = mod


def _round_up(x, m):
    return ((x + m - 1) // m) * m


def _routing(inp, ln_g, ln_b, wg_group, wg_inner):
    """Replicate the reference gating bit-for-bit on jax-cpu.

    Returns gi [N,GK] group ids, gsc [N,GK] group softmax, z [N,D] f32,
    eis/escs: per-group inner top-k ids/softmax ([N,EK] each).
    """
    import jax
    import jax.numpy as jnp

    cpu = jax.devices("cpu")[0]
    with jax.default_device(cpu):
        x = jnp.asarray(np.asarray(inp, np.float32)).reshape(-1, D)
        gl = x @ jnp.asarray(np.asarray(wg_group, np.float32))
        gv, gi = jax.lax.top_k(gl, GK)
        gsc = jax.nn.softmax(gv, axis=-1)
        m = jnp.mean(x, axis=-1, keepdims=True)
        xc = x - m
        v = jnp.mean(xc * xc, axis=-1, keepdims=True)
        z = xc * jax.lax.rsqrt(v + EPS) * jnp.asarray(np.asarray(ln_g, np.float32)) \
            + jnp.asarray(np.asarray(ln_b, np.float32))
        wgi = jnp.asarray(np.asarray(wg_inner, np.float32))
        eis, escs = [], []
        for g in range(G):
            l = z @ wgi[g]
            ev, ei = jax.lax.top_k(l, EK)
            esc = jax.nn.softmax(ev, axis=-1)
            eis.append(np.asarray(ei))
            escs.append(np.asarray(esc))
    return np.asarray(gi), np.asarray(gsc), np.asarray(z), eis, escs


def _build_nc(Cs, has_b1=False):
    """Build the SPMD Bass program for per-slot capacities Cs (uniform across cores)."""
    import concourse.bass as bass
    import concourse.bacc as bacc
    import concourse.tile as tile
    from concourse import mybir

    f32 = mybir.dt.float32
    fp8 = mybir.dt.float8e4
    Relu = mybir.ActivationFunctionType.Relu
    DR = mybir.MatmulPerfMode.DoubleRow
    max_op = mybir.AluOpType.max
    mult_op = mybir.AluOpType.mult

    CT = int(sum(Cs))
    MAXW = int(max(min(int(c), L1_CHUNK) for c in Cs))
    offs = np.concatenate([[0], np.cumsum(Cs)]).astype(int)

    nc = bacc.Bacc("TRN2", target_bir_lowering=False)
    # all DRAM layouts are partition-major so every DMA is 128 dense lines.
    # w1 is h-half-major ([P, half, dt, H/2]) so slot0's first half (all dt,
    # ht 0..7) is one contiguous-line DMA and layer 1 can start on it.
    zt_d = nc.declare_dram_parameter("zt", [P, DT, CT], fp8, isOutput=False)
    w1_d = nc.declare_dram_parameter("w1", [SLOTS, P, 2 * DT, H // 2], fp8, isOutput=False)
    w2_d = nc.declare_dram_parameter("w2", [SLOTS, P, DT * HT, P], fp8, isOutput=False)
    if has_b1:
        b1_d = nc.declare_dram_parameter("b1", [P, SLOTS * HT], f32, isOutput=False)
    u_d = nc.declare_dram_parameter("u", [P, DT, CT], fp8, isOutput=True)

    with tile.TileContext(nc) as tc:
        with tc.tile_pool(name="consts", bufs=1) as consts, \
             tc.tile_pool(name="hpool", bufs=2) as hpool, \
             tc.tile_pool(name="hpsum", bufs=4, space="PSUM") as hpsum, \
             tc.tile_pool(name="upsum", bufs=3, space="PSUM") as upsum, \
             tc.tile_pool(name="usb", bufs=3) as usb:

            zt_sb = consts.tile([P, DT, CT], fp8, tag="zt")
            zero_sb = consts.tile([P, MAXW], f32, tag="zero")
            sixt_sb = consts.tile([P, MAXW], f32, tag="sixt")
            nc.vector.memset(zero_sb[:, :], 0.0)
            nc.vector.memset(sixt_sb[:, :], 1.0 / SU)
            if has_b1:
                b1_sb = consts.tile([P, SLOTS * HT], f32, tag="b1")
            w1_sb, w2_sb = [], []
            for s in range(SLOTS):
                w1_sb.append(consts.tile([P, 2 * DT, H // 2], fp8, tag=f"w1_{s}", name=f"w1s_{s}"))
                w2_sb.append(consts.tile([P, DT * HT, P], fp8, tag=f"w2_{s}", name=f"w2s_{s}"))

            # ---- resident loads. Measured (v2/v5/v7 traces): data only
            # starts flowing ~9us in (instruction fetch + trigger latency);
            # each HWDGE ring then streams 160-190 GB/s serially in queue
            # order; the SWDGE ring starts later and runs ~3x slower. So:
            # weights + zt go ONLY on the two HWDGE rings, every piece a
            # ~4KB-line half so both rings advance in lock-step with
            # first-use order (h-half of w1 feeds the software-pipelined
            # half-L1 emits below); SWDGE carries only the u stores. A late
            # piece stalls the PE, which also re-throttles the HAM p-state.
            # (No PE warm-up matmuls: measured throttle_active grows with
            # total PE activity -- an activity/power cap -- so dummy matmuls
            # would spend duty budget on zeros.)
            nc.sync.dma_start(zt_sb[:, 0:2, :], zt_d[:, 0:2, :])
            nc.scalar.dma_start(w1_sb[0][:, 0:DT, :], w1_d[0][:, 0:DT, :])
            nc.sync.dma_start(zt_sb[:, 2:4, :], zt_d[:, 2:4, :])
            nc.sync.dma_start(w1_sb[0][:, DT:2 * DT, :], w1_d[0][:, DT:2 * DT, :])
            nc.scalar.dma_start(w2_sb[0][:, 2 * HT:, :], w2_d[0][:, 2 * HT:, :])
            nc.sync.dma_start(w2_sb[0][:, 0:2 * HT, :], w2_d[0][:, 0:2 * HT, :])
            for s in range(1, SLOTS):
                nc.scalar.dma_start(w1_sb[s][:, :, :], w1_d[s][:, :, :])
                nc.sync.dma_start(w2_sb[s][:, :, :], w2_d[s][:, :, :])
            if has_b1:
                nc.scalar.dma_start(b1_sb[:, :], b1_d[:, :])

            # ---- compute, software-pipelined: emit the NEXT chunk's first
            # half of layer 1 before this chunk's layer 2, so the relu
            # engines (DVE/ACT) get ~2us of PE runway and layer-2 matmuls
            # never wait on h (the v7 trace showed 32us of PE sem-waits on
            # the relu producer sems).
            chunks = []
            for s in range(SLOTS):
                C = int(Cs[s])
                off = int(offs[s])
                for c0 in range(0, C, L1_CHUNK):
                    chunks.append((s, off + c0, min(L1_CHUNK, C - c0)))
            h_tiles = {}

            def emit_l1(ci, hts):
                s, base, W = chunks[ci]
                if ci not in h_tiles:
                    h_tiles[ci] = hpool.tile(
                        [P, HT, W], fp8, tag="h", name=f"h_{ci}")
                h_sb = h_tiles[ci]
                for ht in hts:
                    hf, h8 = ht // 8, ht % 8
                    ph = hpsum.tile([P, W], f32, tag="ph", name=f"ph_{ci}_{ht}")
                    for j, dt in enumerate((0, 2)):
                        nc.tensor.matmul(
                            ph[:, :],
                            w1_sb[s][:, hf * DT + dt: hf * DT + dt + 2,
                                     h8 * P:(h8 + 1) * P],
                            zt_sb[:, dt:dt + 2, base:base + W],
                            start=(j == 0),
                            stop=(j == 1),
                            perf_mode=DR,
                        )
                    # relu split: even ht on ACT, odd ht on DVE
                    if has_b1:
                        nc.scalar.activation(
                            h_sb[:, ht, :], ph[:, :], Relu,
                            bias=b1_sb[:, s * HT + ht: s * HT + ht + 1],
                        )
                    elif RELU_SPLIT and ht % 2 == 0:
                        nc.scalar.activation(h_sb[:, ht, :], ph[:, :], Relu)
                    else:
                        nc.vector.tensor_tensor(
                            h_sb[:, ht, :], ph[:, :], zero_sb[:, :W], max_op)

            def emit_l2(ci):
                s, base, W = chunks[ci]
                h_sb = h_tiles[ci]
                u_sb = usb.tile([P, DT, W], fp8, tag="u", name=f"u_{ci}")
                for dt in range(DT):
                    pu = upsum.tile([P, W], f32, tag="pu", name=f"pu_{ci}_{dt}")
                    for j, ht in enumerate(range(0, HT, 2)):
                        nc.tensor.matmul(
                            pu[:, :],
                            w2_sb[s][:, dt * HT + ht: dt * HT + ht + 2, :],
                            h_sb[:, ht:ht + 2, :],
                            start=(j == 0),
                            stop=(j == HT // 2 - 1),
                            perf_mode=DR,
                        )
                    nc.vector.tensor_tensor(
                        u_sb[:, dt, :], pu[:, :], sixt_sb[:, :W], mult_op)
                # one batched output DMA per chunk on the SWDGE path
                nc.gpsimd.dma_start(u_d[:, :, base:base + W], u_sb[:, :, :])

            emit_l1(0, range(HT))
            for ci in range(len(chunks)):
                if ci + 1 < len(chunks):
                    emit_l1(ci + 1, range(0, HT // 2))
                emit_l2(ci)
                if ci + 1 < len(chunks):
                    emit_l1(ci + 1, range(HT // 2, HT))
    nc.compile()
    return nc


def _get_nc(Cs, has_b1):
    key = (tuple(int(c) for c in Cs), bool(has_b1))
    if key not in _nc_cache:
        _nc_cache[key] = _build_nc(key[0], key[1])
    return _nc_cache[key]


def _q8(x, scale, f8):
    """Host-side quantize x*scale to TRN e4m3 (clip to +-240)."""
    return np.clip(np.asarray(x, np.float32) * scale, -F8MAX, F8MAX).astype(f8)


def kernel(inp, ln_g, ln_b, wg_group, wg_inner, W1, b1, W2, b2, gln_g, gln_b):
    global LAST_RESULTS
    import jax
    import jax.numpy as jnp
    import ml_dtypes

    inp = np.asarray(inp)
    in_dtype = inp.dtype
    f8 = ml_dtypes.float8_e4m3

    # ---- 1. routing on host (bit-exact replica of the reference gates)
    gi, gsc, z, eis, escs = _routing(inp, ln_g, ln_b, wg_group, wg_inner)

    # token lists per (g, e)
    tok_lists, scale_lists = {}, {}
    for g in range(G):
        in_g = (gi == g).any(axis=1)
        S_g = np.nonzero(in_g)[0]
        ei, esc = eis[g], escs[g]
        for e in range(E):
            sel = ei[S_g] == e           # [|S_g|, EK]
            has = sel.any(axis=1)
            toks = S_g[has]
            w = (esc[S_g] * sel).sum(axis=1)[has]
            tok_lists[(g, e)] = toks
            scale_lists[(g, e)] = w.astype(np.float32)

    # ---- 2. balanced assignment of the 32 pairs to (core, slot)
    pairs = [(g, e) for g in range(G) for e in range(E)]
    pairs.sort(key=lambda p: -len(tok_lists[p]))
    assign = {}           # (core, slot) -> (g, e)
    Cs = []
    for s in range(SLOTS):
        rank = pairs[s * NCORES:(s + 1) * NCORES]
        Cs.append(max(CAP_GRAN, _round_up(max(len(tok_lists[p]) for p in rank), CAP_GRAN)))
        for c, p in enumerate(rank):
            assign[(c, s)] = p
    CT = int(sum(Cs))
    offs = np.concatenate([[0], np.cumsum(Cs)]).astype(int)

    # ---- 3. build per-core input maps (fp8 with power-of-two scales)
    W1n = np.asarray(W1, np.float32)
    W2n = np.asarray(W2, np.float32)
    b1n = np.asarray(b1, np.float32)
    b2n = np.asarray(b2, np.float32)
    has_b1 = bool(np.any(b1n))
    z_q = _q8(z, SZ, f8)

    in_maps = []
    for c in range(NCORES):
        zt_np = np.zeros((P, DT, CT), f8)
        w1_np = np.empty((SLOTS, P, 2 * DT, H // 2), f8)
        w2_np = np.empty((SLOTS, P, DT * HT, P), f8)
        if has_b1:
            b1_np = np.empty((P, SLOTS * HT), np.float32)
            b1_v = b1_np.reshape(P, SLOTS, HT)
        for s in range(SLOTS):
            g, e = assign[(c, s)]
            toks = tok_lists[(g, e)]
            n = len(toks)
            off = offs[s]
            # z^T tile: [tok, d] -> [p, dt, c]
            zt_np[:, :, off:off + n] = z_q[toks].T.reshape(DT, P, n).transpose(1, 0, 2)
            # W1 [d, h] -> [p, half*DT+dt, h%1024] (h-half-major)
            w1_np[s] = (
                _q8(W1n[g, e], S1, f8)
                .reshape(DT, P, 2, H // 2).transpose(1, 2, 0, 3)
                .reshape(P, 2 * DT, H // 2)
            )
            # W2 [h, d] -> [p, dt, ht, dc] (dt-major) -> [p, dt*ht, dc]
            w2_np[s] = (
                _q8(W2n[g, e], S2, f8)
                .reshape(HT, P, DT, P).transpose(1, 2, 0, 3).reshape(P, DT * HT, P)
            )
            if has_b1:
                b1_v[:, s, :] = (SZ * S1) * b1n[g, e].reshape(HT, P).T
        m = {"zt": zt_np, "w1": w1_np, "w2": w2_np}
        if has_b1:
            m["b1"] = b1_np
        in_maps.append(m)

    # ---- 4. compile + run on the 8 NeuronCores
    _ensure_ntff_hook()
    from concourse.bass_utils import run_bass_kernel_spmd

    nc = _get_nc(Cs, has_b1=has_b1)
    res = run_bass_kernel_spmd(
        nc, in_maps, core_ids=list(range(NCORES)),
        trace=bool(int(os.environ.get("KERNEL_TRACE", "0"))),
    )
    LAST_RESULTS = res

    # ---- 5. host combine
    moe = np.zeros((G, N, D), np.float32)
    for c in range(NCORES):
        # u layout [p, dt, c] -> u^T[d, c] -> [CT, D]; device u = UDQ * u
        u = (
            np.asarray(res.results[c]["u"], np.float32)
            .transpose(1, 0, 2).reshape(D, CT).T
        )
        for s in range(SLOTS):
            g, e = assign[(c, s)]
            toks = tok_lists[(g, e)]
            n = len(toks)
            w = scale_lists[(g, e)]
            contrib = u[offs[s]:offs[s] + n] * (w / UDQ)[:, None] \
                + w[:, None] * b2n[g, e][None, :]
            np.add.at(moe[g], toks, contrib)

    cpu = jax.devices("cpu")[0]
    with jax.default_device(cpu):
        zj = jnp.asarray(z)
        gi_j = jnp.asarray(gi)
        gsc_j = jnp.asarray(gsc)
        gw_dense = jnp.sum(
            jax.nn.one_hot(gi_j, G, dtype=jnp.float32) * gsc_j[..., None], axis=-2
        )  # [N, G]
        out = jnp.zeros((N, D), jnp.float32)
        gg = jnp.asarray(np.asarray(gln_g, np.float32))
        gb = jnp.asarray(np.asarray(gln_b, np.float32))
        for g in range(G):
            t = zj + jnp.asarray(moe[g])
            m = jnp.mean(t, axis=-1, keepdims=True)
            tc_ = t - m
            v = jnp.mean(tc_ * tc_, axis=-1, keepdims=True)
            y = tc_ * jax.lax.rsqrt(v + EPS) * gg[g] + gb[g]
            out = out + gw_dense[:, g:g + 1] * y
        result = np.asarray(out).reshape(B, T, D) + np.asarray(inp, np.float32)

    return result.astype(in_dtype)


# revision 24
# speedup vs baseline: 1.2083x; 1.0278x over previous
"""Trainium2 kernel for nn_CustomizedMoGPositionwiseFF (moe_routing).

Strategy (expert-parallel, per the sharding hint):
  - 32 (group, expert) FFN pairs are sharded across 8 NeuronCores (4 each).
  - Routing (group top-2 gate + per-group inner top-2 gate) is computed on
    host at call time; tokens are dispatched (gathered) per expert into the
    per-core shards -- data-dependent sharding, compiled into the NEFF.
  - Each core runs both FFN matmuls + relu for its 4 experts over the tokens
    routed to them, reading each expert weight exactly once (memory regime).
  - Weights/activations ship as fp8e4 (TRN E4M3, max 240) with power-of-two
    scales; matmuls run in DoubleRow perf mode (2 fp8 k-tiles per pass, 2x
    PE throughput, f32 PSUM accumulation):
        zt = 4*z,  w1q = 8*W1   -> psum1 = 32*(z@W1)
        h  = relu(psum1) fp8    (= 32*relu(z@W1), max ~80 < 240)
        w2q = 16*W2             -> psum2 = 512*(h@W2)
        u  = psum2/16 fp8       (= 32*u, max ~50 < 240); host divides by 32.
  - Host applies the cheap O(N*D) combine: iw/b2 scaling, scatter-add of the
    two expert contributions per (token, group), per-group post-layernorm,
    group top-2 mixture, and the outer residual.

Device layouts are partition-major; the DoubleRow pair dim is always the
middle AP dim ([128, 2, cols]): zt/w1 are dt-paired, h/w2 are ht-paired
(w2 stored dt-major so each output d-tile reads a contiguous slab).
"""

import os
import numpy as np

# Model dims (hardcoded per the contract; match the reference problem)
B, T, D, H = 2, 1024, 512, 2048
G, E, GK, EK = 4, 8, 2, 2
EPS = 1e-5
N = B * T
P = 128
DT = D // P    # 4 d-tiles
HT = H // P    # 16 h-tiles
NCORES = 8
SLOTS = (G * E) // NCORES  # 4 experts per core
CAP_GRAN = 4               # capacity granularity (tokens)
L1_CHUNK = 512             # moving-dim chunk for both layers

# fp8 scales (powers of two; see module docstring)
SZ = 4.0    # z
S1 = 8.0    # W1
S2 = 16.0   # W2
SU = 16.0   # psum2 -> u divisor on device
UDQ = (SZ * S1 * S2) / SU  # device u = UDQ * true u; host divides (=32)
F8MAX = 240.0              # TRN E4M3 max normal (inf at 256)

# 1 = split relu between scalar(ACT) and vector(DVE); 0 = all DVE
RELU_SPLIT = int(os.environ.get("KERNEL_RELU_SPLIT", "1"))

_nc_cache = {}
LAST_RESULTS = None       # test harness can inspect (BassKernelResults)


def _ensure_ntff_hook():
    """Register antenv.axon_hooks with the ctypes NTFF profile hook if the
    container's antenv package lacks it (mirrors trn_agent_boot.trn_boot).
    Makes trace=True work; degrades to hook=None when the .so is absent."""
    try:
        from antenv.axon_hooks import get_axon_ntff_profile_hook  # noqa: F401
        return
    except ImportError:
        pass
    import sys
    import types
    import contextlib
    import ctypes

    mod = types.ModuleType("antenv.axon_hooks")
    _state = {"hook": None}

    def set_axon_ntff_profile_hook(h):
        _state["hook"] = h

    def get_axon_ntff_profile_hook():
        return _state["hook"]

    mod.set_axon_ntff_profile_hook = set_axon_ntff_profile_hook
    mod.get_axon_ntff_profile_hook = get_axon_ntff_profile_hook

    so_path = "/opt/axon/libaxon_pjrt.so"
    hook = None
    if os.path.exists(so_path):
        try:
            lib = ctypes.CDLL(so_path)
            if hasattr(lib, "axon_start_nrt_profile"):
                lib.axon_start_nrt_profile.argtypes = [
                    ctypes.POINTER(ctypes.c_int64), ctypes.c_size_t]
                lib.axon_start_nrt_profile.restype = ctypes.c_int64
                lib.axon_stop_nrt_profile.argtypes = [ctypes.c_char_p]
                lib.axon_stop_nrt_profile.restype = ctypes.c_int64

                @contextlib.contextmanager
                def _hook(output_dir, device_ids):
                    import jax
                    jax.devices()
                    if device_ids:
                        ids = (ctypes.c_int64 * len(device_ids))(*device_ids)
                        rc = lib.axon_start_nrt_profile(ids, len(device_ids))
                    else:
                        rc = lib.axon_start_nrt_profile(None, 0)
                    if rc != 0:
                        raise RuntimeError(f"axon_start_nrt_profile rc={rc}")
                    try:
                        yield
                    finally:
                        n = lib.axon_stop_nrt_profile(str(output_dir).encode())
                        print(f"ntff profile: {n} file(s) -> {output_dir}")

                hook = _hook
        except Exception:
            hook = None
    _state["hook"] = hook
    import antenv
    sys.modules["antenv.axon_hooks"] = mod
    antenv.axon_hooks = mod


def _round_up(x, m):
    return ((x + m - 1) // m) * m


def _routing(inp, ln_g, ln_b, wg_group, wg_inner):
    """Replicate the reference gating bit-for-bit on jax-cpu.

    Returns gi [N,GK] group ids, gsc [N,GK] group softmax, z [N,D] f32,
    eis/escs: per-group inner top-k ids/softmax ([N,EK] each).
    """
    import jax
    import jax.numpy as jnp

    cpu = jax.devices("cpu")[0]
    with jax.default_device(cpu):
        x = jnp.asarray(np.asarray(inp, np.float32)).reshape(-1, D)
        gl = x @ jnp.asarray(np.asarray(wg_group, np.float32))
        gv, gi = jax.lax.top_k(gl, GK)
        gsc = jax.nn.softmax(gv, axis=-1)
        m = jnp.mean(x, axis=-1, keepdims=True)
        xc = x - m
        v = jnp.mean(xc * xc, axis=-1, keepdims=True)
        z = xc * jax.lax.rsqrt(v + EPS) * jnp.asarray(np.asarray(ln_g, np.float32)) \
            + jnp.asarray(np.asarray(ln_b, np.float32))
        wgi = jnp.asarray(np.asarray(wg_inner, np.float32))
        eis, escs = [], []
        for g in range(G):
            l = z @ wgi[g]
            ev, ei = jax.lax.top_k(l, EK)
            esc = jax.nn.softmax(ev, axis=-1)
            eis.append(np.asarray(ei))
            escs.append(np.asarray(esc))
    return np.asarray(gi), np.asarray(gsc), np.asarray(z), eis, escs


def _build_nc(Cs, has_b1=False):
    """Build the SPMD Bass program for per-slot capacities Cs (uniform across cores)."""
    import concourse.bass as bass
    import concourse.bacc as bacc
    import concourse.tile as tile
    from concourse import mybir

    f32 = mybir.dt.float32
    fp8 = mybir.dt.float8e4
    Relu = mybir.ActivationFunctionType.Relu
    DR = mybir.MatmulPerfMode.DoubleRow
    max_op = mybir.AluOpType.max
    mult_op = mybir.AluOpType.mult

    CT = int(sum(Cs))
    MAXW = int(max(min(int(c), L1_CHUNK) for c in Cs))
    offs = np.concatenate([[0], np.cumsum(Cs)]).astype(int)

    nc = bacc.Bacc("TRN2", target_bir_lowering=False)
    # all DRAM layouts are partition-major so every DMA is 128 dense lines.
    # w1 is h-half-major ([P, half, dt, H/2]) so slot0's first half (all dt,
    # ht 0..7) is one contiguous-line DMA and layer 1 can start on it.
    zt_d = nc.declare_dram_parameter("zt", [P, DT, CT], fp8, isOutput=False)
    w1_d = nc.declare_dram_parameter("w1", [SLOTS, P, 2 * DT, H // 2], fp8, isOutput=False)
    w2_d = nc.declare_dram_parameter("w2", [SLOTS, P, DT * HT, P], fp8, isOutput=False)
    if has_b1:
        b1_d = nc.declare_dram_parameter("b1", [P, SLOTS * HT], f32, isOutput=False)
    u_d = nc.declare_dram_parameter("u", [P, DT, CT], fp8, isOutput=True)

    with tile.TileContext(nc) as tc:
        with tc.tile_pool(name="consts", bufs=1) as consts, \
             tc.tile_pool(name="hpool", bufs=2) as hpool, \
             tc.tile_pool(name="hpsum", bufs=4, space="PSUM") as hpsum, \
             tc.tile_pool(name="upsum", bufs=3, space="PSUM") as upsum, \
             tc.tile_pool(name="usb", bufs=3) as usb:

            zt_sb = consts.tile([P, DT, CT], fp8, tag="zt")
            zero_sb = consts.tile([P, MAXW], f32, tag="zero")
            sixt_sb = consts.tile([P, MAXW], f32, tag="sixt")
            nc.vector.memset(zero_sb[:, :], 0.0)
            nc.vector.memset(sixt_sb[:, :], 1.0 / SU)
            if has_b1:
                b1_sb = consts.tile([P, SLOTS * HT], f32, tag="b1")
            w1_sb, w2_sb = [], []
            for s in range(SLOTS):
                w1_sb.append(consts.tile([P, 2 * DT, H // 2], fp8, tag=f"w1_{s}", name=f"w1s_{s}"))
                w2_sb.append(consts.tile([P, DT * HT, P], fp8, tag=f"w2_{s}", name=f"w2s_{s}"))

            # ---- resident loads. Measured (v2/v5/v7 traces): data only
            # starts flowing ~9us in (instruction fetch + trigger latency);
            # each HWDGE ring then streams 160-190 GB/s serially in queue
            # order; the SWDGE ring starts later and runs ~3x slower. So:
            # weights + zt go ONLY on the two HWDGE rings, every piece a
            # ~4KB-line half so both rings advance in lock-step with
            # first-use order (h-half of w1 feeds the software-pipelined
            # half-L1 emits below); SWDGE carries only the u stores. A late
            # piece stalls the PE, which also re-throttles the HAM p-state.
            # (No PE warm-up matmuls: measured throttle_active grows with
            # total PE activity -- an activity/power cap -- so dummy matmuls
            # would spend duty budget on zeros.)
            nc.sync.dma_start(zt_sb[:, 0:2, :], zt_d[:, 0:2, :])
            nc.scalar.dma_start(w1_sb[0][:, 0:DT, :], w1_d[0][:, 0:DT, :])
            nc.sync.dma_start(zt_sb[:, 2:4, :], zt_d[:, 2:4, :])
            nc.sync.dma_start(w1_sb[0][:, DT:2 * DT, :], w1_d[0][:, DT:2 * DT, :])
            # w1s1's first half lands before w2s0: the pipelined L1(slot1)
            # half is emitted BEFORE L2(slot0) on the PE FIFO, so a late
            # w1s1 head-of-line-blocks an otherwise-ready L2(slot0).
            nc.scalar.dma_start(w1_sb[1][:, 0:DT, :], w1_d[1][:, 0:DT, :])
            nc.scalar.dma_start(w2_sb[0][:, 2 * HT:, :], w2_d[0][:, 2 * HT:, :])
            nc.sync.dma_start(w2_sb[0][:, 0:2 * HT, :], w2_d[0][:, 0:2 * HT, :])
            nc.sync.dma_start(w1_sb[1][:, DT:2 * DT, :], w1_d[1][:, DT:2 * DT, :])
            nc.scalar.dma_start(w1_sb[2][:, :, :], w1_d[2][:, :, :])
            nc.sync.dma_start(w2_sb[1][:, :, :], w2_d[1][:, :, :])
            nc.scalar.dma_start(w1_sb[3][:, :, :], w1_d[3][:, :, :])
            nc.sync.dma_start(w2_sb[2][:, :, :], w2_d[2][:, :, :])
            nc.sync.dma_start(w2_sb[3][:, :, :], w2_d[3][:, :, :])
            if has_b1:
                nc.scalar.dma_start(b1_sb[:, :], b1_d[:, :])

            # ---- compute, software-pipelined: emit the NEXT chunk's first
            # half of layer 1 before this chunk's layer 2, so the relu
            # engines (DVE/ACT) get ~2us of PE runway and layer-2 matmuls
            # never wait on h (the v7 trace showed 32us of PE sem-waits on
            # the relu producer sems).
            chunks = []
            for s in range(SLOTS):
                C = int(Cs[s])
                off = int(offs[s])
                for c0 in range(0, C, L1_CHUNK):
                    chunks.append((s, off + c0, min(L1_CHUNK, C - c0)))
            h_tiles = {}

            def emit_l1(ci, hts):
                s, base, W = chunks[ci]
                if ci not in h_tiles:
                    h_tiles[ci] = hpool.tile(
                        [P, HT, W], fp8, tag="h", name=f"h_{ci}")
                h_sb = h_tiles[ci]
                for ht in hts:
                    hf, h8 = ht // 8, ht % 8
                    ph = hpsum.tile([P, W], f32, tag="ph", name=f"ph_{ci}_{ht}")
                    for j, dt in enumerate((0, 2)):
                        nc.tensor.matmul(
                            ph[:, :],
                            w1_sb[s][:, hf * DT + dt: hf * DT + dt + 2,
                                     h8 * P:(h8 + 1) * P],
                            zt_sb[:, dt:dt + 2, base:base + W],
                            start=(j == 0),
                            stop=(j == 1),
                            perf_mode=DR,
                        )
                    # relu split: even ht on ACT, odd ht on DVE
                    if has_b1:
                        nc.scalar.activation(
                            h_sb[:, ht, :], ph[:, :], Relu,
                            bias=b1_sb[:, s * HT + ht: s * HT + ht + 1],
                        )
                    elif RELU_SPLIT and ht % 2 == 0:
                        nc.scalar.activation(h_sb[:, ht, :], ph[:, :], Relu)
                    else:
                        nc.vector.tensor_tensor(
                            h_sb[:, ht, :], ph[:, :], zero_sb[:, :W], max_op)

            def emit_l2(ci):
                s, base, W = chunks[ci]
                h_sb = h_tiles[ci]
                u_sb = usb.tile([P, DT, W], fp8, tag="u", name=f"u_{ci}")
                for dt in range(DT):
                    pu = upsum.tile([P, W], f32, tag="pu", name=f"pu_{ci}_{dt}")
                    for j, ht in enumerate(range(0, HT, 2)):
                        nc.tensor.matmul(
                            pu[:, :],
                            w2_sb[s][:, dt * HT + ht: dt * HT + ht + 2, :],
                            h_sb[:, ht:ht + 2, :],
                            start=(j == 0),
                            stop=(j == HT // 2 - 1),
                            perf_mode=DR,
                        )
                    nc.vector.tensor_tensor(
                        u_sb[:, dt, :], pu[:, :], sixt_sb[:, :W], mult_op)
                # one batched output DMA per chunk on the SWDGE path
                nc.gpsimd.dma_start(u_d[:, :, base:base + W], u_sb[:, :, :])

            emit_l1(0, range(HT))
            for ci in range(len(chunks)):
                if ci + 1 < len(chunks):
                    emit_l1(ci + 1, range(0, HT // 2))
                emit_l2(ci)
                if ci + 1 < len(chunks):
                    emit_l1(ci + 1, range(HT // 2, HT))
    nc.compile()
    return nc


def _get_nc(Cs, has_b1):
    key = (tuple(int(c) for c in Cs), bool(has_b1))
    if key not in _nc_cache:
        _nc_cache[key] = _build_nc(key[0], key[1])
    return _nc_cache[key]


def _q8(x, scale, f8):
    """Host-side quantize x*scale to TRN e4m3 (clip to +-240)."""
    return np.clip(np.asarray(x, np.float32) * scale, -F8MAX, F8MAX).astype(f8)


def kernel(inp, ln_g, ln_b, wg_group, wg_inner, W1, b1, W2, b2, gln_g, gln_b):
    global LAST_RESULTS
    import jax
    import jax.numpy as jnp
    import ml_dtypes

    inp = np.asarray(inp)
    in_dtype = inp.dtype
    f8 = ml_dtypes.float8_e4m3

    # ---- 1. routing on host (bit-exact replica of the reference gates)
    gi, gsc, z, eis, escs = _routing(inp, ln_g, ln_b, wg_group, wg_inner)

    # token lists per (g, e)
    tok_lists, scale_lists = {}, {}
    for g in range(G):
        in_g = (gi == g).any(axis=1)
        S_g = np.nonzero(in_g)[0]
        ei, esc = eis[g], escs[g]
        for e in range(E):
            sel = ei[S_g] == e           # [|S_g|, EK]
            has = sel.any(axis=1)
            toks = S_g[has]
            w = (esc[S_g] * sel).sum(axis=1)[has]
            tok_lists[(g, e)] = toks
            scale_lists[(g, e)] = w.astype(np.float32)

    # ---- 2. balanced assignment of the 32 pairs to (core, slot)
    pairs = [(g, e) for g in range(G) for e in range(E)]
    pairs.sort(key=lambda p: -len(tok_lists[p]))
    assign = {}           # (core, slot) -> (g, e)
    Cs = []
    for s in range(SLOTS):
        rank = pairs[s * NCORES:(s + 1) * NCORES]
        Cs.append(max(CAP_GRAN, _round_up(max(len(tok_lists[p]) for p in rank), CAP_GRAN)))
        for c, p in enumerate(rank):
            assign[(c, s)] = p
    CT = int(sum(Cs))
    offs = np.concatenate([[0], np.cumsum(Cs)]).astype(int)

    # ---- 3. build per-core input maps (fp8 with power-of-two scales)
    W1n = np.asarray(W1, np.float32)
    W2n = np.asarray(W2, np.float32)
    b1n = np.asarray(b1, np.float32)
    b2n = np.asarray(b2, np.float32)
    has_b1 = bool(np.any(b1n))
    z_q = _q8(z, SZ, f8)

    in_maps = []
    for c in range(NCORES):
        zt_np = np.zeros((P, DT, CT), f8)
        w1_np = np.empty((SLOTS, P, 2 * DT, H // 2), f8)
        w2_np = np.empty((SLOTS, P, DT * HT, P), f8)
        if has_b1:
            b1_np = np.empty((P, SLOTS * HT), np.float32)
            b1_v = b1_np.reshape(P, SLOTS, HT)
        for s in range(SLOTS):
            g, e = assign[(c, s)]
            toks = tok_lists[(g, e)]
            n = len(toks)
            off = offs[s]
            # z^T tile: [tok, d] -> [p, dt, c]
            zt_np[:, :, off:off + n] = z_q[toks].T.reshape(DT, P, n).transpose(1, 0, 2)
            # W1 [d, h] -> [p, half*DT+dt, h%1024] (h-half-major)
            w1_np[s] = (
                _q8(W1n[g, e], S1, f8)
                .reshape(DT, P, 2, H // 2).transpose(1, 2, 0, 3)
                .reshape(P, 2 * DT, H // 2)
            )
            # W2 [h, d] -> [p, dt, ht, dc] (dt-major) -> [p, dt*ht, dc]
            w2_np[s] = (
                _q8(W2n[g, e], S2, f8)
                .reshape(HT, P, DT, P).transpose(1, 2, 0, 3).reshape(P, DT * HT, P)
            )
            if has_b1:
                b1_v[:, s, :] = (SZ * S1) * b1n[g, e].reshape(HT, P).T
        m = {"zt": zt_np, "w1": w1_np, "w2": w2_np}
        if has_b1:
            m["b1"] = b1_np
        in_maps.append(m)

    # ---- 4. compile + run on the 8 NeuronCores
    _ensure_ntff_hook()
    from concourse.bass_utils import run_bass_kernel_spmd

    nc = _get_nc(Cs, has_b1=has_b1)
    res = run_bass_kernel_spmd(
        nc, in_maps, core_ids=list(range(NCORES)),
        trace=bool(int(os.environ.get("KERNEL_TRACE", "0"))),
    )
    LAST_RESULTS = res

    # ---- 5. host combine
    moe = np.zeros((G, N, D), np.float32)
    for c in range(NCORES):
        # u layout [p, dt, c] -> u^T[d, c] -> [CT, D]; device u = UDQ * u
        u = (
            np.asarray(res.results[c]["u"], np.float32)
            .transpose(1, 0, 2).reshape(D, CT).T
        )
        for s in range(SLOTS):
            g, e = assign[(c, s)]
            toks = tok_lists[(g, e)]
            n = len(toks)
            w = scale_lists[(g, e)]
            contrib = u[offs[s]:offs[s] + n] * (w / UDQ)[:, None] \
                + w[:, None] * b2n[g, e][None, :]
            np.add.at(moe[g], toks, contrib)

    cpu = jax.devices("cpu")[0]
    with jax.default_device(cpu):
        zj = jnp.asarray(z)
        gi_j = jnp.asarray(gi)
        gsc_j = jnp.asarray(gsc)
        gw_dense = jnp.sum(
            jax.nn.one_hot(gi_j, G, dtype=jnp.float32) * gsc_j[..., None], axis=-2
        )  # [N, G]
        out = jnp.zeros((N, D), jnp.float32)
        gg = jnp.asarray(np.asarray(gln_g, np.float32))
        gb = jnp.asarray(np.asarray(gln_b, np.float32))
        for g in range(G):
            t = zj + jnp.asarray(moe[g])
            m = jnp.mean(t, axis=-1, keepdims=True)
            tc_ = t - m
            v = jnp.mean(tc_ * tc_, axis=-1, keepdims=True)
            y = tc_ * jax.lax.rsqrt(v + EPS) * gg[g] + gb[g]
            out = out + gw_dense[:, g:g + 1] * y
        result = np.asarray(out).reshape(B, T, D) + np.asarray(inp, np.float32)

    return result.astype(in_dtype)
